# revision 7
# baseline (speedup 1.0000x reference)
"""Trainium2 Bass kernel for an nn.Block dense transformer layer.

Reference computation (per batch element b of 8):
    x = x + MHA(LN1(x));  x = x + MLP(LN2(x))
with T=1024 tokens, C=512 channels, H=16 heads (d=32), MLP hidden 2048,
new-gelu (tanh approx), softmax without causal mask.

Sharding: pure data parallelism - each of the 8 NeuronCores processes one
batch element.  No collectives.

On-chip dataflow (per core) uses a transposed activation layout
[feature(partition), token(free)]; every linear is
    out^T[f, t] = sum_c W^T[c, f] * x^T[c, t]
with matmul(lhsT=W^T tile, rhs=x^T tile).

Differences vs the f32 baseline (all validated numerically, rel ~3e-3):
  - Whole pipeline in bf16 (residual stream, LN outputs, all weights);
    PSUM accumulation stays fp32.  N=1024 moving for all bf16 matmuls.
  - LN scale/shift absorbed into the following matmul weights on the host
    (ln1 -> qkv, ln2 -> fc); v-bias pushed through attention into proj_b.
    LN on chip is just (x - mu) * rstd.
  - Softmax exp split across TWO engines: half the tiles evacuate PSUM via
    ScalarE table Exp, the other half via a Schraudolph bit-trick exp on
    VectorE (tensor_scalar fp32->int16 whose bit pattern IS the bf16 exp).
  - ACT table sets: phases ordered so only 2 table switches per iteration
    (ln/exp set for LN+softmax, gelu set for the MLP).
  - LayerNorm stats via replicated-ones matmul (partition reduction on PE);
    rstd = exp(-0.5*ln(var+eps)) stays on the ln/exp table set.
  - Attention scores computed transposed S^T[k, q] per head with 4-head
    row-group packing; A^T V col-group packed; softmax denominators via
    ones-matmul, all as in the baseline.
"""

import sys

if "/opt/trn_rl_repo" not in sys.path:
    sys.path.insert(0, "/opt/trn_rl_repo")

import math
from contextlib import ExitStack

import ml_dtypes
import numpy as np

import concourse.bass as bass
import concourse.mybir as mybir
import concourse.tile as tile
from concourse import bacc
from concourse import bass_utils

F32 = mybir.dt.float32
F32R = mybir.dt.float32r
BF16 = mybir.dt.bfloat16
I16 = mybir.dt.int16
AF = mybir.ActivationFunctionType
OP = mybir.AluOpType

N_CORES = 8
T = 1024  # tokens
C = 512  # channels
H = 16  # heads
D = 32  # head dim
FF = 2048  # mlp hidden
CT = C // 128  # channel partition tiles (4)
TT = T // 128  # token partition tiles (8)
FT = FF // 128  # mlp hidden partition tiles (16)
NQ = T // 512  # token (query) 512-chunks (2)
G = H // 4  # head groups of 4 (4)
EPS = 1e-5
SCALE = 1.0 / math.sqrt(D)
# Schraudolph exp in bf16-bit domain: bits = round(x * 2^7/ln2 + 127*2^7)
SCH_A = (2.0 ** 7) / math.log(2.0) * SCALE  # applied to raw (unscaled) scores
SCH_B = 127.0 * 2.0 ** 7
GELU_FUNC = AF.Gelu_apprx_tanh
# exp-engine balance: pairs (of 2 sc tiles) whose ACT tile is ALSO sent to
# DVE.  pair_idx runs 0..63; pair_idx % 32 < EXTRA_DVE -> both tiles on DVE.
EXTRA_DVE = 0


class _NS:
    pass


def emit_prep(ctx, nc, tc, io, repeat_tag=""):
    """Allocate persistent tiles, load weights/consts, create SBUF pools."""
    P = _NS()

    wpool = ctx.enter_context(tc.tile_pool(name="w" + repeat_tag, bufs=1))

    def single(shape, dtype, tag):
        return wpool.tile(shape, dtype, tag=tag, name=tag)

    w_qkv = [single([128, 3 * C], BF16, f"wqkv{k}") for k in range(CT)]
    w_proj = [single([128, C], BF16, f"wproj{k}") for k in range(CT)]
    w_fc = [single([128, FF], BF16, f"wfc{k}") for k in range(CT)]
    w_cproj = [single([128, C], BF16, f"wcproj{k}") for k in range(FT)]
    for k in range(CT):
        nc.sync.dma_start(out=w_qkv[k], in_=io["wqkvT"][128 * k : 128 * (k + 1), :])
        nc.sync.dma_start(out=w_proj[k], in_=io["wprojT"][128 * k : 128 * (k + 1), :])
        nc.sync.dma_start(out=w_fc[k], in_=io["wfcT"][128 * k : 128 * (k + 1), :])
    for k in range(FT):
        nc.sync.dma_start(out=w_cproj[k], in_=io["wcprojT"][128 * k : 128 * (k + 1), :])

    # bias columns: tile[p, m] = vec[m*128 + p]
    def colmat(dram_ap, ntiles, tag):
        t = single([128, ntiles], F32, tag)
        nc.sync.dma_start(out=t, in_=dram_ap.transpose([1, 0]))
        return t

    b_qk = colmat(io["bqk"], 8, "bqk")
    b_proj = colmat(io["bproj"], CT, "bproj")
    b_fc = colmat(io["bfc"], FT, "bfc")
    b_cproj = colmat(io["bcproj"], CT, "bcproj")

    ones_f = single([128, 128], BF16, "ones_f")
    nc.vector.memset(ones_f, 1.0)
    ones_b = single([128, 32], BF16, "ones_b")
    nc.vector.memset(ones_b, 1.0)
    eps_t = single([128, 1], F32, "eps_t")
    nc.vector.memset(eps_t, EPS)

    # persistent activation tiles
    x_t = [single([128, T], BF16, f"xT{k}") for k in range(CT)]  # residual
    a_t = [single([128, T], BF16, f"aT{k}") for k in range(CT)]  # ln out
    q_t = [single([128, T], BF16, f"qT{g}") for g in range(G)]
    k_t = [single([128, T], BF16, f"kT{g}") for g in range(G)]
    v_sb = [single([128, C], BF16, f"v{t}") for t in range(TT)]
    av_t = [single([128, T], BF16, f"avT{g}") for g in range(G)]

    tmp = ctx.enter_context(tc.tile_pool(name="tmp" + repeat_tag, bufs=2))
    atp = ctx.enter_context(tc.tile_pool(name="atp" + repeat_tag, bufs=18))
    gtp = ctx.enter_context(tc.tile_pool(name="gtp" + repeat_tag, bufs=17))

    for name in ("w_qkv", "w_proj", "w_fc", "w_cproj", "b_qk", "b_proj", "b_fc",
                 "b_cproj", "ones_f", "ones_b", "eps_t", "x_t", "a_t", "q_t",
                 "k_t", "v_sb", "av_t", "tmp", "atp", "gtp"):
        setattr(P, name, locals()[name])
    return P


def emit_body(nc, tc, io, P, repeat_tag=""):
    """Per-iteration work: load x, compute the block, store y."""
    xT, yT = io["xT"], io["yT"]
    (w_qkv, w_proj, w_fc, w_cproj, b_qk, b_proj, b_fc, b_cproj, ones_f, ones_b,
     eps_t, x_t, a_t, q_t, k_t, v_sb, av_t, tmp, atp, gtp) = (
        P.w_qkv, P.w_proj, P.w_fc, P.w_cproj, P.b_qk, P.b_proj, P.b_fc,
        P.b_cproj, P.ones_f, P.ones_b, P.eps_t, P.x_t, P.a_t, P.q_t, P.k_t,
        P.v_sb, P.av_t, P.tmp, P.atp, P.gtp)

    for k in range(CT):
        nc.sync.dma_start(out=x_t[k], in_=xT[128 * k : 128 * (k + 1), :])

    ctx2 = ExitStack()
    ctx2.__enter__()
    big = ctx2.enter_context(
        tc.tile_pool(name="big" + repeat_tag, bufs=2, space="PSUM")
    )
    stp = ctx2.enter_context(
        tc.tile_pool(name="stp" + repeat_tag, bufs=2, space="PSUM")
    )
    avp = ctx2.enter_context(
        tc.tile_pool(name="avp" + repeat_tag, bufs=1, space="PSUM")
    )
    dnp = ctx2.enter_context(
        tc.tile_pool(name="dnp" + repeat_tag, bufs=1, space="PSUM")
    )

    # ------------- LayerNorm (no scale/shift - absorbed into weights) ------
    def layernorm(src_tiles, dst_tiles):
        for nt in range(NQ):
            cols = slice(512 * nt, 512 * (nt + 1))
            musum = stp.tile([128, 512], F32, tag="st", name="ln_mu")
            sqsum = stp.tile([128, 512], F32, tag="st", name="ln_sq")
            for k in range(CT):
                sq = tmp.tile([128, 512], BF16, tag="sq", name="sq")
                nc.vector.tensor_tensor(
                    out=sq, in0=src_tiles[k][:, cols], in1=src_tiles[k][:, cols],
                    op=OP.mult,
                )
                nc.tensor.matmul(
                    out=musum, lhsT=ones_f, rhs=src_tiles[k][:, cols],
                    start=(k == 0), stop=(k == CT - 1),
                )
                nc.tensor.matmul(
                    out=sqsum, lhsT=ones_f, rhs=sq,
                    start=(k == 0), stop=(k == CT - 1),
                )
            mu = tmp.tile([128, 512], BF16, tag="mu", name="mu")
            ex2 = tmp.tile([128, 512], F32, tag="ex2", name="ex2")
            var = tmp.tile([128, 512], F32, tag="var", name="var")
            rstd = tmp.tile([128, 512], BF16, tag="rstd", name="rstd")
            nc.vector.tensor_scalar_mul(out=mu, in0=musum, scalar1=1.0 / C)
            nc.vector.tensor_scalar_mul(out=ex2, in0=sqsum, scalar1=1.0 / C)
            # var = E[x^2] - mu^2   (var reused as mu^2 scratch)
            nc.vector.tensor_tensor(out=var, in0=mu, in1=mu, op=OP.mult)
            nc.vector.tensor_tensor(out=var, in0=ex2, in1=var, op=OP.subtract)
            # rstd = exp(-0.5 * ln(var + eps))  (stays on the ln/exp table set)
            nc.scalar.activation(out=var, in_=var, func=AF.Ln, bias=eps_t, scale=1.0)
            nc.scalar.activation(out=rstd, in_=var, func=AF.Exp, bias=0.0, scale=-0.5)
            for k in range(CT):
                dst = dst_tiles[k][:, cols]
                zc = tmp.tile([128, 512], BF16, tag="zc", name="zc")
                nc.vector.tensor_tensor(
                    out=zc, in0=src_tiles[k][:, cols], in1=mu, op=OP.subtract
                )
                nc.vector.tensor_tensor(out=dst, in0=zc, in1=rstd, op=OP.mult)

    # ============================ LN1 =====================================
    layernorm(x_t, a_t)

    # ============================ QKV =====================================
    # q^T, k^T (feature on partitions), bf16 + bias via ACT Copy evac
    for m in range(8):  # 8 feature tiles: 4 q, 4 k
        dst = q_t[m] if m < 4 else k_t[m - 4]
        ps = big.tile([128, 1024], F32, tag="big", name="qk_ps")
        for nt in range(NQ):
            cs = slice(512 * nt, 512 * (nt + 1))
            for k in range(CT):
                nc.tensor.matmul(
                    out=ps[:, cs],
                    lhsT=w_qkv[k][:, 128 * m : 128 * (m + 1)],
                    rhs=a_t[k][:, cs],
                    start=(k == 0),
                    stop=(k == CT - 1),
                )
        nc.scalar.activation(
            out=dst, in_=ps, func=AF.Identity, bias=b_qk[:, m : m + 1], scale=1.0
        )
    # v natural layout [token, feature]: lhsT = a^T tile, rhs = wv^T (no bias)
    for t in range(TT):
        ps = stp.tile([128, 512], F32, tag="st", name="v_ps")
        for k in range(CT):
            nc.tensor.matmul(
                out=ps,
                lhsT=a_t[k][:, 128 * t : 128 * (t + 1)],
                rhs=w_qkv[k][:, 2 * C : 3 * C],
                start=(k == 0),
                stop=(k == CT - 1),
            )
        nc.vector.tensor_copy(out=v_sb[t], in_=ps)

    # ============================ Attention ===============================
    pair_idx = 0
    for qc in range(NQ):
        qs = slice(512 * qc, 512 * (qc + 1))
        for g in range(G):
            av_ps = avp.tile([128, 512], F32, tag="av", name="av_ps")
            dn_ps = dnp.tile([128, 512], F32, tag="dn", name="dn_ps")
            for half in range(2):
                kts = range(4 * half, 4 * half + 4)
                at = {}
                for kt in kts:
                    sc = [
                        big.tile([128, 1024], F32, tag="big", name="sc_ps")
                        for _ in range(2)
                    ]
                    for c in range(4):
                        pr = slice(32 * c, 32 * (c + 1))
                        nc.tensor.matmul(
                            out=sc[c // 2][:, 512 * (c % 2) : 512 * (c % 2 + 1)],
                            lhsT=k_t[g][pr, 128 * kt : 128 * (kt + 1)],
                            rhs=q_t[g][pr, qs],
                            start=True,
                            stop=True,
                            tile_position=(32 * c, 0),
                        )
                    both_dve = (pair_idx % 32) < EXTRA_DVE
                    pair_idx += 1
                    for i in range(2):
                        a = atp.tile([128, 1024], I16, tag="at", name="at")
                        if i == 1 or both_dve:
                            # Schraudolph: int16 bits = raw*SCH_A + SCH_B,
                            # bit pattern read back as bf16 = exp(SCALE*raw)
                            nc.vector.tensor_scalar(
                                out=a, in0=sc[i],
                                scalar1=SCH_A, scalar2=SCH_B,
                                op0=OP.mult, op1=OP.add,
                            )
                        else:
                            nc.scalar.activation(
                                out=a.bitcast(BF16), in_=sc[i],
                                func=AF.Exp, bias=0.0, scale=SCALE,
                            )
                        at[i, kt] = a.bitcast(BF16)
                for kt in kts:
                    for c in range(4):
                        rhs = at[c // 2, kt][:, 512 * (c % 2) : 512 * (c % 2 + 1)]
                        nc.tensor.matmul(
                            out=av_ps[32 * c : 32 * (c + 1), :],
                            lhsT=v_sb[kt][:, 32 * (4 * g + c) : 32 * (4 * g + c) + 32],
                            rhs=rhs,
                            start=(kt == 0),
                            stop=(kt == TT - 1),
                            tile_position=(0, 32 * c),
                            skip_group_check=True,
                        )
                        nc.tensor.matmul(
                            out=dn_ps[32 * c : 32 * (c + 1), :],
                            lhsT=ones_b,
                            rhs=rhs,
                            start=(kt == 0),
                            stop=(kt == TT - 1),
                            tile_position=(0, 32 * c),
                            skip_group_check=True,
                        )
            # denominators replicated within each 32-partition block
            dr = tmp.tile([128, 512], F32, tag="dr", name="dr")
            nc.vector.reciprocal_approx_fast(out=dr, in_=dn_ps)
            nc.vector.tensor_tensor(
                out=av_t[g][:, qs], in0=av_ps, in1=dr, op=OP.mult
            )

    # ===================== proj + residual (full T) =======================
    for m in range(CT):
        ps = big.tile([128, 1024], F32, tag="big", name="proj_ps")
        for nt in range(NQ):
            cs = slice(512 * nt, 512 * (nt + 1))
            for g in range(G):
                nc.tensor.matmul(
                    out=ps[:, cs],
                    lhsT=w_proj[g][:, 128 * m : 128 * (m + 1)],
                    rhs=av_t[g][:, cs],
                    start=(g == 0),
                    stop=(g == G - 1),
                )
        nc.vector.scalar_tensor_tensor(
            out=x_t[m],
            in0=ps,
            scalar=b_proj[:, m : m + 1],
            in1=x_t[m],
            op0=OP.add,
            op1=OP.add,
        )

    # ============================ LN2 =====================================
    layernorm(x_t, a_t)

    # ====================== fc + gelu (full T) ============================
    g_tiles = []
    for m in range(FT):
        ps = big.tile([128, 1024], F32, tag="big", name="fc_ps")
        for nt in range(NQ):
            cs = slice(512 * nt, 512 * (nt + 1))
            for k in range(CT):
                nc.tensor.matmul(
                    out=ps[:, cs],
                    lhsT=w_fc[k][:, 128 * m : 128 * (m + 1)],
                    rhs=a_t[k][:, cs],
                    start=(k == 0),
                    stop=(k == CT - 1),
                )
        gt = gtp.tile([128, 1024], BF16, tag="gt", name="gt")
        g_tiles.append(gt)
        nc.scalar.activation(
            out=gt, in_=ps, func=GELU_FUNC, bias=b_fc[:, m : m + 1], scale=1.0
        )

    # =================== cproj + bias + residual ==========================
    for m in range(CT):
        ps = big.tile([128, 1024], F32, tag="big", name="cproj_ps")
        for nt in range(NQ):
            cs = slice(512 * nt, 512 * (nt + 1))
            for k in range(FT):
                nc.tensor.matmul(
                    out=ps[:, cs],
                    lhsT=w_cproj[k][:, 128 * m : 128 * (m + 1)],
                    rhs=g_tiles[k][:, cs],
                    start=(k == 0),
                    stop=(k == FT - 1),
                )
        nc.vector.scalar_tensor_tensor(
            out=x_t[m],
            in0=ps,
            scalar=b_cproj[:, m : m + 1],
            in1=x_t[m],
            op0=OP.add,
            op1=OP.add,
        )
    ctx2.__exit__(None, None, None)

    for m in range(CT):
        nc.sync.dma_start(out=yT[128 * m : 128 * (m + 1), :], in_=x_t[m])


def emit_block(ctx, nc, tc, io, repeat_tag=""):
    P = emit_prep(ctx, nc, tc, io, repeat_tag)
    emit_body(nc, tc, io, P, repeat_tag)


def declare_io(nc):
    def inp(name, shape, dtype=F32):
        return nc.dram_tensor(name, shape, dtype, kind="ExternalInput").ap()

    io = {
        "xT": inp("xT", [C, T], BF16),
        "wqkvT": inp("wqkvT", [C, 3 * C], BF16),
        "wprojT": inp("wprojT", [C, C], BF16),
        "wfcT": inp("wfcT", [C, FF], BF16),
        "wcprojT": inp("wcprojT", [FF, C], BF16),
        "bqk": inp("bqk", [8, 128]),
        "bproj": inp("bproj", [CT, 128]),
        "bfc": inp("bfc", [FT, 128]),
        "bcproj": inp("bcproj", [CT, 128]),
        "yT": nc.dram_tensor("yT", [C, T], BF16, kind="ExternalOutput").ap(),
    }
    return io


def build(num_devices=N_CORES):
    nc = bacc.Bacc(
        "TRN2", target_bir_lowering=False, debug=False, num_devices=num_devices
    )
    io = declare_io(nc)
    with tile.TileContext(nc) as tc, ExitStack() as ctx:
        emit_block(ctx, nc, tc, io)
    nc.compile()
    return nc


def host_inputs(x_b, attn_w, attn_b, proj_w, proj_b, fc_w, fc_b, cproj_w, cproj_b,
                ln1_w, ln1_b, ln2_w, ln2_b):
    """Per-core input dict for batch element x_b [T, C], with the LN scale/
    shift and v-bias absorbed into the adjacent linear weights."""
    bf = ml_dtypes.bfloat16
    f = np.float32
    f64 = np.float64
    attn_w2 = attn_w.astype(f64) * ln1_w.astype(f64)[None, :]
    attn_b2 = attn_b.astype(f64) + attn_w.astype(f64) @ ln1_b.astype(f64)
    fc_w2 = fc_w.astype(f64) * ln2_w.astype(f64)[None, :]
    fc_b2 = fc_b.astype(f64) + fc_w.astype(f64) @ ln2_b.astype(f64)
    proj_b2 = proj_b.astype(f64) + proj_w.astype(f64) @ attn_b2[2 * C :]
    return {
        "xT": np.ascontiguousarray(x_b.T).astype(bf),
        "wqkvT": np.ascontiguousarray(attn_w2.T).astype(bf),
        "wprojT": np.ascontiguousarray(proj_w.T).astype(bf),
        "wfcT": np.ascontiguousarray(fc_w2.T).astype(bf),
        "wcprojT": np.ascontiguousarray(cproj_w.T).astype(bf),
        "bqk": np.ascontiguousarray(attn_b2[: 2 * C].reshape(8, 128), dtype=f),
        "bproj": np.ascontiguousarray(proj_b2.reshape(CT, 128), dtype=f),
        "bfc": np.ascontiguousarray(fc_b2.reshape(FT, 128), dtype=f),
        "bcproj": np.ascontiguousarray(cproj_b.reshape(CT, 128), dtype=f),
    }


_CACHED_NC = None


def kernel(x, ln1_w, ln1_b, attn_w, attn_b, proj_w, proj_b,
           ln2_w, ln2_b, fc_w, fc_b, cproj_w, cproj_b):
    global _CACHED_NC
    x = np.asarray(x)
    B = x.shape[0]
    assert B == N_CORES and x.shape[1] == T and x.shape[2] == C
    if _CACHED_NC is None:
        _CACHED_NC = build()
    nc = _CACHED_NC
    args = [np.asarray(a) for a in (attn_w, attn_b, proj_w, proj_b, fc_w, fc_b,
                                    cproj_w, cproj_b, ln1_w, ln1_b, ln2_w, ln2_b)]
    (attn_w, attn_b, proj_w, proj_b, fc_w, fc_b,
     cproj_w, cproj_b, ln1_w, ln1_b, ln2_w, ln2_b) = args
    in_maps = [
        host_inputs(x[b], attn_w, attn_b, proj_w, proj_b, fc_w, fc_b,
                    cproj_w, cproj_b, ln1_w, ln1_b, ln2_w, ln2_b)
        for b in range(B)
    ]
    res = bass_utils.run_bass_kernel_spmd(
        nc, in_maps, core_ids=list(range(N_CORES))
    )
    out = np.empty((B, T, C), np.float32)
    for b in range(B):
        out[b] = res.results[b]["yT"].astype(np.float32).T
    return out


# revision 11
# speedup vs baseline: 1.5415x; 1.5415x over previous
"""Trainium2 Bass kernel for an nn.Block dense transformer layer.

Reference computation (per batch element b of 8):
    x = x + MHA(LN1(x));  x = x + MLP(LN2(x))
with T=1024 tokens, C=512 channels, H=16 heads (d=32), MLP hidden 2048,
new-gelu (tanh approx), softmax without causal mask.

Sharding: pure data parallelism - each of the 8 NeuronCores processes one
batch element.  No collectives.

On-chip dataflow (per core) uses a transposed activation layout
[feature(partition), token(free)]; every linear is
    out^T[f, t] = sum_c W^T[c, f] * x^T[c, t]
with matmul(lhsT=W^T tile, rhs=x^T tile).

Differences vs the f32 baseline (all validated numerically, rel ~3e-3):
  - Whole pipeline in bf16 (residual stream, LN outputs, all weights);
    PSUM accumulation stays fp32.  N=1024 moving for all bf16 matmuls.
  - LN scale/shift absorbed into the following matmul weights on the host
    (ln1 -> qkv, ln2 -> fc); v-bias pushed through attention into proj_b.
    LN on chip is just (x - mu) * rstd.
  - Softmax exp split across TWO engines: half the tiles evacuate PSUM via
    ScalarE table Exp, the other half via a Schraudolph bit-trick exp on
    VectorE (tensor_scalar fp32->int16 whose bit pattern IS the bf16 exp).
  - ACT table sets: phases ordered so only 2 table switches per iteration
    (ln/exp set for LN+softmax, gelu set for the MLP).
  - LayerNorm stats via replicated-ones matmul (partition reduction on PE);
    rstd = exp(-0.5*ln(var+eps)) stays on the ln/exp table set.
  - Attention scores computed transposed S^T[k, q] per head with 4-head
    row-group packing; A^T V col-group packed; softmax denominators via
    ones-matmul, all as in the baseline.
"""

import sys

if "/opt/trn_rl_repo" not in sys.path:
    sys.path.insert(0, "/opt/trn_rl_repo")

import math
from contextlib import ExitStack

import ml_dtypes
import numpy as np

import concourse.bass as bass
import concourse.mybir as mybir
import concourse.tile as tile
from concourse import bacc
from concourse import bass_utils

F32 = mybir.dt.float32
F32R = mybir.dt.float32r
BF16 = mybir.dt.bfloat16
I16 = mybir.dt.int16
AF = mybir.ActivationFunctionType
OP = mybir.AluOpType

N_CORES = 8
T = 1024  # tokens
C = 512  # channels
H = 16  # heads
D = 32  # head dim
FF = 2048  # mlp hidden
CT = C // 128  # channel partition tiles (4)
TT = T // 128  # token partition tiles (8)
FT = FF // 128  # mlp hidden partition tiles (16)
NQ = T // 512  # token (query) 512-chunks (2)
G = H // 4  # head groups of 4 (4)
EPS = 1e-5
SCALE = 1.0 / math.sqrt(D)
# Schraudolph exp in bf16-bit domain: bits = round(x * 2^7/ln2 + 127*2^7)
SCH_A = (2.0 ** 7) / math.log(2.0) * SCALE  # applied to raw (unscaled) scores
SCH_B = 127.0 * 2.0 ** 7
GELU_FUNC = AF.Gelu_apprx_tanh
# exp-engine balance: pairs (of 2 sc tiles) whose ACT tile is ALSO sent to
# DVE.  pair_idx runs 0..63; pair_idx % 32 < EXTRA_DVE -> both tiles on DVE.
EXTRA_DVE = 0


class _NS:
    pass


def emit_prep(ctx, nc, tc, io, repeat_tag=""):
    """Allocate persistent tiles, load weights/consts, create SBUF pools."""
    P = _NS()

    wpool = ctx.enter_context(tc.tile_pool(name="w" + repeat_tag, bufs=1))

    def single(shape, dtype, tag):
        return wpool.tile(shape, dtype, tag=tag, name=tag)

    w_qkv = [single([128, 3 * C], BF16, f"wqkv{k}") for k in range(CT)]
    w_proj = [single([128, C], BF16, f"wproj{k}") for k in range(CT)]
    w_fc = [single([128, FF], BF16, f"wfc{k}") for k in range(CT)]
    w_cproj = [single([128, C], BF16, f"wcproj{k}") for k in range(FT)]
    for k in range(CT):
        nc.sync.dma_start(out=w_qkv[k], in_=io["wqkvT"][128 * k : 128 * (k + 1), :])
        nc.sync.dma_start(out=w_proj[k], in_=io["wprojT"][128 * k : 128 * (k + 1), :])
        nc.sync.dma_start(out=w_fc[k], in_=io["wfcT"][128 * k : 128 * (k + 1), :])
    for k in range(FT):
        nc.sync.dma_start(out=w_cproj[k], in_=io["wcprojT"][128 * k : 128 * (k + 1), :])

    # bias columns: tile[p, m] = vec[m*128 + p]
    def colmat(dram_ap, ntiles, tag):
        t = single([128, ntiles], F32, tag)
        nc.sync.dma_start(out=t, in_=dram_ap.transpose([1, 0]))
        return t

    b_qk = colmat(io["bqk"], 8, "bqk")
    b_proj = colmat(io["bproj"], CT, "bproj")
    b_fc = colmat(io["bfc"], FT, "bfc")
    b_cproj = colmat(io["bcproj"], CT, "bcproj")

    ones_f = single([128, 128], BF16, "ones_f")
    nc.vector.memset(ones_f, 1.0)
    ones_b = single([128, 32], BF16, "ones_b")
    nc.vector.memset(ones_b, 1.0)
    eps_t = single([128, 1], F32, "eps_t")
    nc.vector.memset(eps_t, EPS)

    # persistent activation tiles
    x_t = [single([128, T], BF16, f"xT{k}") for k in range(CT)]  # residual
    a_t = [single([128, T], BF16, f"aT{k}") for k in range(CT)]  # ln out
    q_t = [single([128, T], BF16, f"qT{g}") for g in range(G)]
    k_t = [single([128, T], BF16, f"kT{g}") for g in range(G)]
    v_sb = [single([128, C], BF16, f"v{t}") for t in range(TT)]
    av_t = [single([128, T], BF16, f"avT{g}") for g in range(G)]

    tmp = ctx.enter_context(tc.tile_pool(name="tmp" + repeat_tag, bufs=2))
    atp = ctx.enter_context(tc.tile_pool(name="atp" + repeat_tag, bufs=18))
    gtp = ctx.enter_context(tc.tile_pool(name="gtp" + repeat_tag, bufs=17))

    # PSUM: one rotating pool of 3x [128,1024] f32 (6 banks) + av + dn (2)
    big = ctx.enter_context(
        tc.tile_pool(name="big" + repeat_tag, bufs=3, space="PSUM")
    )
    avp = ctx.enter_context(
        tc.tile_pool(name="avp" + repeat_tag, bufs=1, space="PSUM")
    )
    dnp = ctx.enter_context(
        tc.tile_pool(name="dnp" + repeat_tag, bufs=1, space="PSUM")
    )

    for name in ("w_qkv", "w_proj", "w_fc", "w_cproj", "b_qk", "b_proj", "b_fc",
                 "b_cproj", "ones_f", "ones_b", "eps_t", "x_t", "a_t", "q_t",
                 "k_t", "v_sb", "av_t", "tmp", "atp", "gtp", "big", "avp", "dnp"):
        setattr(P, name, locals()[name])
    return P


def emit_body(nc, tc, io, P, repeat_tag=""):
    """Per-iteration work: load x, compute the block, store y."""
    xT, yT = io["xT"], io["yT"]
    (w_qkv, w_proj, w_fc, w_cproj, b_qk, b_proj, b_fc, b_cproj, ones_f, ones_b,
     eps_t, x_t, a_t, q_t, k_t, v_sb, av_t, tmp, atp, gtp, big, avp, dnp) = (
        P.w_qkv, P.w_proj, P.w_fc, P.w_cproj, P.b_qk, P.b_proj, P.b_fc,
        P.b_cproj, P.ones_f, P.ones_b, P.eps_t, P.x_t, P.a_t, P.q_t, P.k_t,
        P.v_sb, P.av_t, P.tmp, P.atp, P.gtp, P.big, P.avp, P.dnp)

    for k in range(CT):
        nc.sync.dma_start(out=x_t[k], in_=xT[128 * k : 128 * (k + 1), :])

    # ------------- LayerNorm (no scale/shift - absorbed into weights) ------
    def layernorm(src_tiles, dst_tiles):
        for nt in range(NQ):
            cols = slice(512 * nt, 512 * (nt + 1))
            musum = big.tile([128, 1024], F32, tag="big", name="ln_mu")[:, :512]
            sqsum = big.tile([128, 1024], F32, tag="big", name="ln_sq")[:, :512]
            for k in range(CT):
                sq = tmp.tile([128, 512], BF16, tag="sq", name="sq")
                nc.vector.tensor_tensor(
                    out=sq, in0=src_tiles[k][:, cols], in1=src_tiles[k][:, cols],
                    op=OP.mult,
                )
                nc.tensor.matmul(
                    out=musum, lhsT=ones_f, rhs=src_tiles[k][:, cols],
                    start=(k == 0), stop=(k == CT - 1),
                )
                nc.tensor.matmul(
                    out=sqsum, lhsT=ones_f, rhs=sq,
                    start=(k == 0), stop=(k == CT - 1),
                )
            mu = tmp.tile([128, 512], BF16, tag="mu", name="mu")
            ex2 = tmp.tile([128, 512], F32, tag="ex2", name="ex2")
            var = tmp.tile([128, 512], F32, tag="var", name="var")
            rstd = tmp.tile([128, 512], BF16, tag="rstd", name="rstd")
            nc.vector.tensor_scalar_mul(out=mu, in0=musum, scalar1=1.0 / C)
            nc.vector.tensor_scalar_mul(out=ex2, in0=sqsum, scalar1=1.0 / C)
            # var = E[x^2] - mu^2   (var reused as mu^2 scratch)
            nc.vector.tensor_tensor(out=var, in0=mu, in1=mu, op=OP.mult)
            nc.vector.tensor_tensor(out=var, in0=ex2, in1=var, op=OP.subtract)
            # rstd = exp(-0.5 * ln(var + eps))  (stays on the ln/exp table set)
            nc.scalar.activation(out=var, in_=var, func=AF.Ln, bias=eps_t, scale=1.0)
            nc.scalar.activation(out=rstd, in_=var, func=AF.Exp, bias=0.0, scale=-0.5)
            for k in range(CT):
                dst = dst_tiles[k][:, cols]
                zc = tmp.tile([128, 512], BF16, tag="zc", name="zc")
                nc.vector.tensor_tensor(
                    out=zc, in0=src_tiles[k][:, cols], in1=mu, op=OP.subtract
                )
                nc.vector.tensor_tensor(out=dst, in0=zc, in1=rstd, op=OP.mult)

    # ============================ LN1 =====================================
    layernorm(x_t, a_t)

    # ============================ QKV =====================================
    # q^T, k^T (feature on partitions), bf16 + bias via ACT Copy evac
    for m in range(8):  # 8 feature tiles: 4 q, 4 k
        dst = q_t[m] if m < 4 else k_t[m - 4]
        ps = big.tile([128, 1024], F32, tag="big", name="qk_ps")
        for nt in range(NQ):
            cs = slice(512 * nt, 512 * (nt + 1))
            for k in range(CT):
                nc.tensor.matmul(
                    out=ps[:, cs],
                    lhsT=w_qkv[k][:, 128 * m : 128 * (m + 1)],
                    rhs=a_t[k][:, cs],
                    start=(k == 0),
                    stop=(k == CT - 1),
                )
        nc.scalar.activation(
            out=dst, in_=ps, func=AF.Identity, bias=b_qk[:, m : m + 1], scale=1.0
        )
    # v natural layout [token, feature]: lhsT = a^T tile, rhs = wv^T (no bias)
    for t in range(TT):
        ps = big.tile([128, 1024], F32, tag="big", name="v_ps")[:, :512]
        for k in range(CT):
            nc.tensor.matmul(
                out=ps,
                lhsT=a_t[k][:, 128 * t : 128 * (t + 1)],
                rhs=w_qkv[k][:, 2 * C : 3 * C],
                start=(k == 0),
                stop=(k == CT - 1),
            )
        nc.vector.tensor_copy(out=v_sb[t], in_=ps)

    # ============================ Attention ===============================
    pair_idx = 0
    for qc in range(NQ):
        qs = slice(512 * qc, 512 * (qc + 1))
        for g in range(G):
            av_ps = avp.tile([128, 512], F32, tag="av", name="av_ps")
            dn_ps = dnp.tile([128, 512], F32, tag="dn", name="dn_ps")
            for half in range(2):
                kts = range(4 * half, 4 * half + 4)
                at = {}
                for kt in kts:
                    sc = [
                        big.tile([128, 1024], F32, tag="big", name="sc_ps")
                        for _ in range(2)
                    ]
                    for c in range(4):
                        pr = slice(32 * c, 32 * (c + 1))
                        nc.tensor.matmul(
                            out=sc[c // 2][:, 512 * (c % 2) : 512 * (c % 2 + 1)],
                            lhsT=k_t[g][pr, 128 * kt : 128 * (kt + 1)],
                            rhs=q_t[g][pr, qs],
                            start=True,
                            stop=True,
                            tile_position=(32 * c, 0),
                        )
                    both_dve = (pair_idx % 32) < EXTRA_DVE
                    pair_idx += 1
                    for i in range(2):
                        a = atp.tile([128, 1024], I16, tag="at", name="at")
                        if i == 1 or both_dve:
                            # Schraudolph: int16 bits = raw*SCH_A + SCH_B,
                            # bit pattern read back as bf16 = exp(SCALE*raw)
                            nc.vector.tensor_scalar(
                                out=a, in0=sc[i],
                                scalar1=SCH_A, scalar2=SCH_B,
                                op0=OP.mult, op1=OP.add,
                            )
                        else:
                            nc.scalar.activation(
                                out=a.bitcast(BF16), in_=sc[i],
                                func=AF.Exp, bias=0.0, scale=SCALE,
                            )
                        at[i, kt] = a.bitcast(BF16)
                for kt in kts:
                    for c in range(4):
                        rhs = at[c // 2, kt][:, 512 * (c % 2) : 512 * (c % 2 + 1)]
                        nc.tensor.matmul(
                            out=av_ps[32 * c : 32 * (c + 1), :],
                            lhsT=v_sb[kt][:, 32 * (4 * g + c) : 32 * (4 * g + c) + 32],
                            rhs=rhs,
                            start=(kt == 0),
                            stop=(kt == TT - 1),
                            tile_position=(0, 32 * c),
                            skip_group_check=True,
                        )
                        nc.tensor.matmul(
                            out=dn_ps[32 * c : 32 * (c + 1), :],
                            lhsT=ones_b,
                            rhs=rhs,
                            start=(kt == 0),
                            stop=(kt == TT - 1),
                            tile_position=(0, 32 * c),
                            skip_group_check=True,
                        )
            # denominators replicated within each 32-partition block
            dr = tmp.tile([128, 512], F32, tag="dr", name="dr")
            nc.vector.reciprocal_approx_fast(out=dr, in_=dn_ps)
            nc.vector.tensor_tensor(
                out=av_t[g][:, qs], in0=av_ps, in1=dr, op=OP.mult
            )

    # ===================== proj + residual (full T) =======================
    for m in range(CT):
        ps = big.tile([128, 1024], F32, tag="big", name="proj_ps")
        for nt in range(NQ):
            cs = slice(512 * nt, 512 * (nt + 1))
            for g in range(G):
                nc.tensor.matmul(
                    out=ps[:, cs],
                    lhsT=w_proj[g][:, 128 * m : 128 * (m + 1)],
                    rhs=av_t[g][:, cs],
                    start=(g == 0),
                    stop=(g == G - 1),
                )
        nc.vector.scalar_tensor_tensor(
            out=x_t[m],
            in0=ps,
            scalar=b_proj[:, m : m + 1],
            in1=x_t[m],
            op0=OP.add,
            op1=OP.add,
        )

    # ============================ LN2 =====================================
    layernorm(x_t, a_t)

    # ====================== fc + gelu (full T) ============================
    g_tiles = []
    for m in range(FT):
        ps = big.tile([128, 1024], F32, tag="big", name="fc_ps")
        for nt in range(NQ):
            cs = slice(512 * nt, 512 * (nt + 1))
            for k in range(CT):
                nc.tensor.matmul(
                    out=ps[:, cs],
                    lhsT=w_fc[k][:, 128 * m : 128 * (m + 1)],
                    rhs=a_t[k][:, cs],
                    start=(k == 0),
                    stop=(k == CT - 1),
                )
        gt = gtp.tile([128, 1024], BF16, tag="gt", name="gt")
        g_tiles.append(gt)
        nc.scalar.activation(
            out=gt, in_=ps, func=GELU_FUNC, bias=b_fc[:, m : m + 1], scale=1.0
        )

    # =================== cproj + bias + residual ==========================
    for m in range(CT):
        ps = big.tile([128, 1024], F32, tag="big", name="cproj_ps")
        for nt in range(NQ):
            cs = slice(512 * nt, 512 * (nt + 1))
            for k in range(FT):
                nc.tensor.matmul(
                    out=ps[:, cs],
                    lhsT=w_cproj[k][:, 128 * m : 128 * (m + 1)],
                    rhs=g_tiles[k][:, cs],
                    start=(k == 0),
                    stop=(k == FT - 1),
                )
        nc.vector.scalar_tensor_tensor(
            out=x_t[m],
            in0=ps,
            scalar=b_cproj[:, m : m + 1],
            in1=x_t[m],
            op0=OP.add,
            op1=OP.add,
        )

    for m in range(CT):
        nc.sync.dma_start(out=yT[128 * m : 128 * (m + 1), :], in_=x_t[m])


def emit_block(ctx, nc, tc, io, repeat_tag=""):
    P = emit_prep(ctx, nc, tc, io, repeat_tag)
    emit_body(nc, tc, io, P, repeat_tag)


def declare_io(nc):
    def inp(name, shape, dtype=F32):
        return nc.dram_tensor(name, shape, dtype, kind="ExternalInput").ap()

    io = {
        "xT": inp("xT", [C, T], BF16),
        "wqkvT": inp("wqkvT", [C, 3 * C], BF16),
        "wprojT": inp("wprojT", [C, C], BF16),
        "wfcT": inp("wfcT", [C, FF], BF16),
        "wcprojT": inp("wcprojT", [FF, C], BF16),
        "bqk": inp("bqk", [8, 128]),
        "bproj": inp("bproj", [CT, 128]),
        "bfc": inp("bfc", [FT, 128]),
        "bcproj": inp("bcproj", [CT, 128]),
        "yT": nc.dram_tensor("yT", [C, T], BF16, kind="ExternalOutput").ap(),
    }
    return io


def build(num_devices=N_CORES):
    nc = bacc.Bacc(
        "TRN2", target_bir_lowering=False, debug=False, num_devices=num_devices
    )
    io = declare_io(nc)
    with tile.TileContext(nc) as tc, ExitStack() as ctx:
        emit_block(ctx, nc, tc, io)
    nc.compile()
    return nc


def host_inputs(x_b, attn_w, attn_b, proj_w, proj_b, fc_w, fc_b, cproj_w, cproj_b,
                ln1_w, ln1_b, ln2_w, ln2_b):
    """Per-core input dict for batch element x_b [T, C], with the LN scale/
    shift and v-bias absorbed into the adjacent linear weights."""
    bf = ml_dtypes.bfloat16
    f = np.float32
    f64 = np.float64
    attn_w2 = attn_w.astype(f64) * ln1_w.astype(f64)[None, :]
    attn_b2 = attn_b.astype(f64) + attn_w.astype(f64) @ ln1_b.astype(f64)
    fc_w2 = fc_w.astype(f64) * ln2_w.astype(f64)[None, :]
    fc_b2 = fc_b.astype(f64) + fc_w.astype(f64) @ ln2_b.astype(f64)
    proj_b2 = proj_b.astype(f64) + proj_w.astype(f64) @ attn_b2[2 * C :]
    return {
        "xT": np.ascontiguousarray(x_b.T).astype(bf),
        "wqkvT": np.ascontiguousarray(attn_w2.T).astype(bf),
        "wprojT": np.ascontiguousarray(proj_w.T).astype(bf),
        "wfcT": np.ascontiguousarray(fc_w2.T).astype(bf),
        "wcprojT": np.ascontiguousarray(cproj_w.T).astype(bf),
        "bqk": np.ascontiguousarray(attn_b2[: 2 * C].reshape(8, 128), dtype=f),
        "bproj": np.ascontiguousarray(proj_b2.reshape(CT, 128), dtype=f),
        "bfc": np.ascontiguousarray(fc_b2.reshape(FT, 128), dtype=f),
        "bcproj": np.ascontiguousarray(cproj_b.reshape(CT, 128), dtype=f),
    }


_CACHED_NC = None


def kernel(x, ln1_w, ln1_b, attn_w, attn_b, proj_w, proj_b,
           ln2_w, ln2_b, fc_w, fc_b, cproj_w, cproj_b):
    global _CACHED_NC
    x = np.asarray(x)
    B = x.shape[0]
    assert B == N_CORES and x.shape[1] == T and x.shape[2] == C
    if _CACHED_NC is None:
        _CACHED_NC = build()
    nc = _CACHED_NC
    args = [np.asarray(a) for a in (attn_w, attn_b, proj_w, proj_b, fc_w, fc_b,
                                    cproj_w, cproj_b, ln1_w, ln1_b, ln2_w, ln2_b)]
    (attn_w, attn_b, proj_w, proj_b, fc_w, fc_b,
     cproj_w, cproj_b, ln1_w, ln1_b, ln2_w, ln2_b) = args
    in_maps = [
        host_inputs(x[b], attn_w, attn_b, proj_w, proj_b, fc_w, fc_b,
                    cproj_w, cproj_b, ln1_w, ln1_b, ln2_w, ln2_b)
        for b in range(B)
    ]
    res = bass_utils.run_bass_kernel_spmd(
        nc, in_maps, core_ids=list(range(N_CORES))
    )
    out = np.empty((B, T, C), np.float32)
    for b in range(B):
        out[b] = res.results[b]["yT"].astype(np.float32).T
    return out
